# revision 32
# baseline (speedup 1.0000x reference)
"""LocalGraphMessageBlock TRN2 kernel (v2).

Math (per chunk of C=512 tokens, H=256 features, offsets 1,2,4,8):
  h_in = LN(h);  per offset o and direction, uniform full-width N=512:
    z = P[t] + Q[t+qo] + CF @ e20[t+eo]   (qo=+o fwd / -o rev, eo=0 / -o)
    CF carries: +-C_unit, C_dist, BIG*(mask-1) and b1 (via const ones row),
    so gelu(z)=0 exactly on invalid/out-of-range edges.
    agg += gelu(z) @ w2            (PSUM accumulation across all 8 dirs)
  agg += b2 (x) deg                (outer product, deg = #valid edges per dst)
  h2 = h + agg;  out = (h2 + MLP(LN(h2))) * valid

Implementation notes:
  - Feature-major on chip ([128 feat, 512 tok]); tokens enter/leave via
    transpose-DMA access patterns (512B descriptors), no PE transposes.
  - z assembled in PSUM: e-matmul (start) + identity-matmul of tz (stop),
    where tz = P + Q_window is ONE [128,4,512] 3D-AP DVE add per dir-off.
    Gelu reads PSUM directly on ACT ([128,1024] per half).
  - P = A^T h_in, Q = B^T h_in precomputed per chunk (A,B = halves of w1);
    b1 folded into CF's const-ones row (e20 row 96).
  - rsqrt via DVE bit-hack + 1 Newton step (ACT stays on the gelu table).
  - PSUM: agg 2 banks + ze 2x2 banks + mmps 1 + rows 1 = 8.

Data-parallel over the chunk dim N: 256 chunks / 8 cores = 32 chunks each,
same NEFF, per-core input slices.
"""
import json

import numpy as np
import ml_dtypes

BF = ml_dtypes.bfloat16

N_TOT, C, H = 256, 512, 256
OFFSETS = (1, 2, 4, 8)
N_CORES = 8
CPC = N_TOT // N_CORES  # chunks per core
BIG = 30000.0
EPS = 1e-5
MAGIC = 0x5F3759DF

# ---------------------------------------------------------------------------
# Walrus workaround: this container's walrus accepts at most ONE sync-wait
# command per instruction; Tile emits more. Split excess onto preceding
# NoOps on the same engine (engine queues are in-order, so this is
# equivalent gating).
# ---------------------------------------------------------------------------
_patched = False


def _split_sync_waits(bir_json: bytes, maxw: int = 1) -> bytes:
    m = json.loads(bir_json)
    cnt = 0
    changed = False
    for f in m.get("functions", []):
        for blk in f.get("blocks", []):
            newins = []
            for ins in blk.get("instructions", []):
                si = ins.get("sync_info")
                if si:
                    waits = si.get("on_wait") or []
                    if len(waits) > maxw:
                        changed = True
                        si["on_wait"] = waits[-maxw:]
                        extra = waits[:-maxw]
                        for i in range(0, len(extra), maxw):
                            cnt += 1
                            newins.append({
                                "debug": ins.get("debug", 0),
                                "engine": ins["engine"],
                                "ins": [], "outs": [],
                                "name": f"{ins['name']}-ws{cnt}",
                                "opcode": "NoOp",
                                "sync_info": {"on_update": [],
                                              "on_wait": extra[i:i + maxw]},
                            })
                newins.append(ins)
            blk["instructions"] = newins
    return json.dumps(m).encode() if changed else bir_json


def _install_patch():
    global _patched
    if _patched:
        return
    import concourse.bass_utils as bu
    import concourse.bass2jax as b2j

    orig = bu.compile_bir_kernel

    def patched(bir_json, tmpdir, neff_name="file.neff"):
        return orig(_split_sync_waits(bir_json), tmpdir, neff_name)

    bu.compile_bir_kernel = patched
    b2j.compile_bir_kernel = patched
    _patched = True


# ---------------------------------------------------------------------------
# Bass kernel builder
# ---------------------------------------------------------------------------
_nc_cache = {}


def _build(n_chunks):
    import concourse.bass as bass
    import concourse.tile as tile
    from concourse import mybir

    f32 = mybir.dt.float32
    bf16 = mybir.dt.bfloat16
    f8 = mybir.dt.float8e4
    i32 = mybir.dt.int32
    u8 = mybir.dt.uint8
    AF = mybir.ActivationFunctionType
    ALU = mybir.AluOpType
    DR = mybir.MatmulPerfMode.DoubleRow

    nc = bass.Bass("TRN2")

    # ---- dram I/O ----
    h_d = nc.dram_tensor("h", [n_chunks, C, H], f32, kind="ExternalInput")
    xyz_d = nc.dram_tensor("xyz", [n_chunks, C, 3], f32, kind="ExternalInput")
    val_d = nc.dram_tensor("valid", [n_chunks, C], u8, kind="ExternalInput")
    out_d = nc.dram_tensor("out", [n_chunks, C, H], f32, kind="ExternalOutput")

    def din(name, shape, dt=None):
        return nc.dram_tensor(name, shape, dt or bf16, kind="ExternalInput")

    A_d = din("A", [2, 128, 512])       # w1[:256] k-halves  (lhsT blocks)
    B_d = din("B", [2, 128, 512])       # w1[256:512]
    W2_d = din("W2", [128, 1024])       # w2 k-blocks: [:, k*256:(k+1)*256]
    U1_d = din("U1", [2, 128, 1024])
    U2_d = din("U2", [128, 2048])       # u2 k-blocks: [:, k*256:(k+1)*256]
    CF_d = din("CF", [128, 4096])       # e-matmul lhsT blocks (oi,dir,m)
    bu1c_d = din("bu1c", [128, 8], f32)
    bu2c_d = din("bu2c", [128, 2], f32)
    b2r_d = din("b2r", [1, 256])
    lnw_d = din("lnw", [1, 256])
    lnb_d = din("lnb", [1, 256])
    lnuw_d = din("lnuw", [1, 256])
    lnub_d = din("lnub", [1, 256])
    onesH_d = din("onesH", [128, 1])    # 1/H
    ones4_d = din("ones4", [4, 1])
    seldsq_d = din("seldsq", [12, 4])
    selbc_d = din("selbc", [4, 12])
    onesr_d = din("onesr", [1, 512])
    ident_d = din("ident", [128, 128], f32)
    identb_d = din("identb", [128, 128])
    seed_d = din("seed", [128, 24], i32)

    from contextlib import ExitStack
    with tile.TileContext(nc) as tc, ExitStack() as ctx:
        cp = ctx.enter_context(tc.tile_pool(name="consts", bufs=1))
        ld = ctx.enter_context(tc.tile_pool(name="loads", bufs=2))
        wk = ctx.enter_context(tc.tile_pool(name="work", bufs=2))
        sm = ctx.enter_context(tc.tile_pool(name="small", bufs=2))
        pagg = ctx.enter_context(tc.tile_pool(name="pagg", bufs=1,
                                              space="PSUM"))
        pze = ctx.enter_context(tc.tile_pool(name="pze", bufs=2,
                                             space="PSUM"))
        pmm = ctx.enter_context(tc.tile_pool(name="pmm", bufs=2, space="PSUM"))
        prw = ctx.enter_context(tc.tile_pool(name="prw", bufs=2, space="PSUM"))

        # ---- load constants ----
        def cload(dram, shape, dt=None, name=None):
            t = cp.tile(shape, dt or bf16, name=name, tag=name)
            nc.sync.dma_start(t, dram[tuple(slice(None) for _ in shape)])
            return t

        A0 = cp.tile([128, 512], bf16); nc.sync.dma_start(A0, A_d[0])
        A1 = cp.tile([128, 512], bf16); nc.sync.dma_start(A1, A_d[1])
        B0 = cp.tile([128, 512], bf16); nc.sync.dma_start(B0, B_d[0])
        B1 = cp.tile([128, 512], bf16); nc.sync.dma_start(B1, B_d[1])
        W2 = cload(W2_d, [128, 1024], name="W2")
        U1a = cp.tile([128, 1024], bf16); nc.sync.dma_start(U1a, U1_d[0])
        U1b = cp.tile([128, 1024], bf16); nc.sync.dma_start(U1b, U1_d[1])
        U2 = cload(U2_d, [128, 2048], name="U2")
        CF = cload(CF_d, [128, 4096], name="CF")
        bu1c = cload(bu1c_d, [128, 8], f32, name="bu1c")
        bu2c = cload(bu2c_d, [128, 2], f32, name="bu2c")
        b2r = cload(b2r_d, [1, 256], name="b2r")
        lnw = cload(lnw_d, [1, 256], name="lnw")
        lnb = cload(lnb_d, [1, 256], name="lnb")
        lnuw = cload(lnuw_d, [1, 256], name="lnuw")
        lnub = cload(lnub_d, [1, 256], name="lnub")
        onesH = cload(onesH_d, [128, 1], name="onesH")
        ones4 = cload(ones4_d, [4, 1], name="ones4")
        seldsq = cload(seldsq_d, [12, 4], name="seldsq")
        selbc = cload(selbc_d, [4, 12], name="selbc")
        onesr = cload(onesr_d, [1, 512], name="onesr")
        ident = cload(ident_d, [128, 128], f32, name="ident")
        identb = cload(identb_d, [128, 128], name="identb")
        seed = cload(seed_d, [128, 24], i32, name="seed")

        # e20 rows: unit 0:12 (3/oi), dist 32:36, mask 64:68, ones 96.
        # Two buffers (chunk parity). Init all to -1 (pad cols' mask=-1 ->
        # z-=BIG -> gelu=0; dead rows hit zero CF rows), then ones row.
        e20s = []
        for pbuf in range(2):
            e = cp.tile([128, 520], bf16, name=f"e20_{pbuf}", tag=f"e20_{pbuf}")
            nc.vector.tensor_scalar(out=e, in0=CF[:, 0:520],
                                    scalar1=0.0, scalar2=-1.0,
                                    op0=ALU.mult, op1=ALU.add)
            nc.vector.memset(e[96:97], 1.0)
            e20s.append(e)
        Qs = []
        for pbuf in range(2):
            q = cp.tile([128, 2064], bf16, name=f"Qt_{pbuf}", tag=f"Qt_{pbuf}")
            nc.vector.memset(q[:, 0:8], 0.0)
            nc.vector.memset(q[:, 2056:2064], 0.0)
            Qs.append(q)

        def rsqrt_rows(rows, r, tag, want_f32=False):
            """rows: [r, 512] f32 sbuf (positive) -> [r, 512] bf16 1/sqrt
            (optionally also f32) via bit-hack + 1 Newton step."""
            w = 4 * r
            rt = prw.tile([128, w], f32, name="rt", tag="rows")
            for g in range(4):
                nc.tensor.transpose(rt[:, g * r:(g + 1) * r],
                                    rows[:, g * 128:(g + 1) * 128],
                                    ident[0:r, 0:r])
            x = sm.tile([128, w], f32, name=f"nrx{tag}", tag=f"nrx{tag}")
            nc.vector.tensor_copy(x, rt)
            yi = sm.tile([128, w], i32, name=f"nry{tag}", tag=f"nry{tag}")
            nc.vector.tensor_scalar(out=yi, in0=x.bitcast(i32), scalar1=1,
                                    scalar2=None, op0=ALU.logical_shift_right)
            nc.vector.tensor_sub(yi, seed[:, 0:w], yi)
            y0 = yi.bitcast(f32)
            y = sm.tile([128, w], f32, name=f"nryy{tag}", tag=f"nryy{tag}")
            t = sm.tile([128, w], f32, name=f"nrt{tag}", tag=f"nrt{tag}")
            nc.vector.tensor_mul(t, y0, y0)
            nc.vector.tensor_mul(t, t, x)
            nc.vector.tensor_scalar(out=t, in0=t, scalar1=-0.5,
                                    scalar2=1.5, op0=ALU.mult, op1=ALU.add)
            nc.vector.tensor_mul(y, y0, t)
            rp = prw.tile([r, 512], f32, name="rp", tag="rows")
            for g in range(4):
                nc.tensor.transpose(rp[:, g * 128:(g + 1) * 128],
                                    y[:, g * r:(g + 1) * r], ident)
            outb = sm.tile([r, 512], bf16, name=f"nro{tag}", tag=f"nro{tag}")
            nc.vector.tensor_copy(outb, rp)
            if not want_f32:
                return outb, None
            outf = sm.tile([r, 512], f32, name=f"nrof{tag}", tag=f"nrof{tag}")
            nc.vector.tensor_copy(outf, rp)
            return outb, outf

        def ln_fm(hfm, wrow, brow, tag):
            """Feature-major layernorm of hfm [128,1024] bf16 -> [128,1024]
            bf16 (fh blocks of 512 tokens side by side)."""
            mu_ps = prw.tile([1, 512], f32, name="mu_ps", tag="rows")
            m2_ps = prw.tile([1, 512], f32, name="m2_ps", tag="rows")
            x2 = wk.tile([128, 1024], bf16, name=f"x2{tag}", tag="x2", bufs=1)
            nc.gpsimd.tensor_mul(x2, hfm, hfm)
            for fh in range(2):
                nc.tensor.matmul(mu_ps, onesH, hfm[:, fh * 512:
                                                   (fh + 1) * 512],
                                 start=(fh == 0), stop=(fh == 1))
                nc.tensor.matmul(m2_ps, onesH, x2[:, fh * 512:
                                                  (fh + 1) * 512],
                                 start=(fh == 0), stop=(fh == 1))
            mu_row = sm.tile([1, 512], f32, name=f"mur{tag}", tag=f"mur{tag}")
            nc.scalar.copy(mu_row, mu_ps)
            mumu = sm.tile([1, 512], f32, name=f"mumu{tag}", tag=f"mumu{tag}")
            nc.scalar.activation(mumu, mu_ps, AF.Square)
            vare = sm.tile([1, 512], f32, name=f"var{tag}", tag=f"var{tag}")
            nc.vector.scalar_tensor_tensor(out=vare, in0=m2_ps,
                                           scalar=EPS, in1=mumu, op0=ALU.add,
                                           op1=ALU.subtract)
            rstd, _ = rsqrt_rows(vare, 1, tag)
            sh_row = sm.tile([1, 512], bf16, name=f"shr{tag}", tag=f"shr{tag}")
            nc.vector.scalar_tensor_tensor(out=sh_row, in0=mu_row, scalar=-1.0,
                                           in1=rstd, op0=ALU.mult,
                                           op1=ALU.mult)
            o = wk.tile([128, 1024], bf16, name=f"ln{tag}", tag=f"ln{tag}",
                        bufs=2)
            for fh in range(2):
                arep = prw.tile([128, 512], f32, name="arep", tag="rows")
                nc.tensor.matmul(arep, wrow[:, fh * 128:(fh + 1) * 128], rstd,
                                 start=True, stop=True)
                brep = prw.tile([128, 512], f32, name="brep", tag="rows")
                nc.tensor.matmul(brep, wrow[:, fh * 128:(fh + 1) * 128],
                                 sh_row, start=True, stop=False)
                nc.tensor.matmul(brep, brow[:, fh * 128:(fh + 1) * 128],
                                 onesr, start=False, stop=True)
                sl = o[:, fh * 512:(fh + 1) * 512]
                nc.vector.tensor_mul(sl, hfm[:, fh * 512:(fh + 1) * 512],
                                     arep)
                nc.vector.tensor_add(sl, sl, brep)
            return o

        def stage_a(ci):
            """Loads, in-transposes, LN1, edge features, P/Q. PE-light —
            scheduled to overlap the previous chunk's stage_b."""
            e20 = e20s[ci % 2]
            Q = Qs[ci % 2]
            # ---------------- loads (token-major h) ----------------
            ht = ld.tile([128, 1024], f32, name="ht", tag="ht")
            hb = h_d[ci]
            nc.sync.dma_start(
                ht.rearrange("p (i f) -> p i f", i=4),
                bass.AP(tensor=hb.tensor, offset=hb.offset,
                        ap=[[256, 128], [128 * 256, 4], [1, 256]]))
            xyzp = sm.tile([3, 520], f32, name="xyzp", tag="xyzp")
            nc.vector.memset(xyzp, 0.0)
            nc.sync.dma_start(xyzp[:, 0:512],
                              xyz_d[ci].rearrange("t k -> k t"))
            vbase = val_d[ci]
            vr_u8 = sm.tile([4, 512], u8, name="vru", tag="vru")
            nc.sync.dma_start(
                vr_u8, bass.AP(tensor=vbase.tensor, offset=vbase.offset,
                               ap=[[0, 4], [1, 512]]))
            vrf = sm.tile([4, 512], f32, name="vrf", tag="vrf")
            nc.vector.tensor_copy(vrf, vr_u8)
            vrs_u8 = sm.tile([4, 512], u8, name="vrsu", tag="vrsu")
            nc.vector.memset(vrs_u8, 0)
            for oi, off in enumerate(OFFSETS):
                nc.sync.dma_start(vrs_u8[oi:oi + 1, 0:C - off],
                                  val_d[ci, off:C][None, :])
            vrsf = sm.tile([4, 512], f32, name="vrsf", tag="vrsf")
            nc.vector.tensor_copy(vrsf, vrs_u8)
            vrp_u8 = sm.tile([4, 512], u8, name="vrpu", tag="vrpu")
            nc.vector.memset(vrp_u8, 0)
            for oi, off in enumerate(OFFSETS):
                nc.sync.dma_start(vrp_u8[oi:oi + 1, off:C],
                                  val_d[ci, 0:C - off][None, :])
            vrpf = sm.tile([4, 512], f32, name="vrpf", tag="vrpf")
            nc.vector.tensor_copy(vrpf, vrp_u8)
            vcol_u8 = sm.tile([128, 4], u8, name="vcu", tag="vcu")
            nc.sync.dma_start(
                vcol_u8, bass.AP(tensor=vbase.tensor, offset=vbase.offset,
                                 ap=[[1, 128], [128, 4]]))
            vcolf = sm.tile([128, 4], f32, name="vcf", tag="vcf")
            nc.vector.tensor_copy(vcolf, vcol_u8)

            # ---------------- h -> feature-major (bf16) ----------------
            hfm = wk.tile([128, 1024], bf16, name="hfm", tag="hfm")
            for g in range(4):
                for fh in range(2):
                    tp = prw.tile([128, 128], f32, name="tp", tag="rows")
                    nc.tensor.transpose(
                        tp,
                        ht[:, g * 256 + fh * 128:g * 256 + (fh + 1) * 128],
                        ident)
                    nc.scalar.copy(
                        hfm[:, fh * 512 + g * 128:fh * 512 + (g + 1) * 128],
                        tp)

            # ---------------- LN1 ----------------
            hin = ln_fm(hfm, lnw, lnb, "a")

            # ---------------- edge features ----------------
            delta = sm.tile([12, 512], f32, name="delta", tag="delta")
            for oi, off in enumerate(OFFSETS):
                dlo = sm.tile([3, 512], f32, name=f"dlo{oi}", tag=f"dlo{oi}")
                nc.vector.tensor_sub(dlo, xyzp[:, off:off + 512],
                                     xyzp[:, 0:512])
                nc.sync.dma_start(delta[3 * oi:3 * oi + 3], dlo)
            dsq = sm.tile([12, 512], bf16, name="dsq", tag="dsq")
            nc.gpsimd.tensor_mul(dsq, delta, delta)
            d2_ps = prw.tile([4, 512], f32, name="d2_ps", tag="rows")
            nc.tensor.matmul(d2_ps, seldsq, dsq, start=True, stop=True)
            R = sm.tile([4, 512], f32, name="Rrows", tag="Rrows")
            nc.vector.tensor_scalar(out=R[0:4], in0=d2_ps,
                                    scalar1=1e-12, scalar2=None, op0=ALU.max)

            m_all = sm.tile([4, 512], bf16, name="mall", tag="mall")
            nc.gpsimd.tensor_mul(m_all, vrf, vrsf)
            nc.vector.tensor_scalar(out=e20[64:68, 8:520], in0=m_all,
                                    scalar1=1.0, scalar2=None,
                                    op0=ALU.subtract)

            rsq, rsqf = rsqrt_rows(R, 4, "e", want_f32=True)

            invrep = prw.tile([12, 512], f32, name="invrep", tag="rows")
            nc.tensor.matmul(invrep, selbc, rsq[0:4], start=True, stop=True)
            nc.vector.tensor_mul(e20[0:12, 8:520], delta, invrep)
            nc.vector.tensor_mul(e20[32:36, 8:520], R[0:4], rsqf[0:4])

            # ---------------- degree / agg init ----------------
            mrev = sm.tile([4, 512], bf16, name="mrev", tag="mrev")
            nc.gpsimd.tensor_mul(mrev, vrf, vrpf)
            deg_ps = prw.tile([1, 512], f32, name="deg_ps", tag="rows")
            nc.tensor.matmul(deg_ps, ones4, m_all, start=True, stop=False)
            nc.tensor.matmul(deg_ps, ones4, mrev, start=False, stop=True)
            deg_row = sm.tile([1, 512], bf16, name="degr", tag="degr")
            nc.scalar.copy(deg_row, deg_ps)

            # ---------------- P, Q ----------------
            P = wk.tile([128, 2048], bf16, name="P", tag="P")
            for m in range(4):
                pq = pmm.tile([128, 512], f32, name="mmps", tag="mmps")
                nc.tensor.matmul(pq, A0[:, m * 128:(m + 1) * 128],
                                 hin[:, 0:512], start=True, stop=False)
                nc.tensor.matmul(pq, A1[:, m * 128:(m + 1) * 128],
                                 hin[:, 512:1024], start=False, stop=True)
                nc.vector.tensor_copy(P[:, m * 512:(m + 1) * 512], pq)
                pq2 = pmm.tile([128, 512], f32, name="mmps", tag="mmps")
                nc.tensor.matmul(pq2, B0[:, m * 128:(m + 1) * 128],
                                 hin[:, 0:512], start=True, stop=False)
                nc.tensor.matmul(pq2, B1[:, m * 128:(m + 1) * 128],
                                 hin[:, 512:1024], start=False, stop=True)
                nc.vector.tensor_copy(Q[:, 8 + m * 512:8 + (m + 1) * 512],
                                      pq2)
            return dict(hfm=hfm, P=P, deg_row=deg_row, vcolf=vcolf)

        def stage_b(ci, st):
            """agg init, message loop, update MLP, masked store."""
            e20 = e20s[ci % 2]
            Q = Qs[ci % 2]
            hfm = st["hfm"]
            P = st["P"]
            vcolf = st["vcolf"]

            agg = pagg.tile([128, 1024], f32, name="agg", tag="agg")
            for fh in range(2):
                nc.tensor.matmul(agg[:, fh * 512:(fh + 1) * 512],
                                 b2r[:, fh * 128:(fh + 1) * 128],
                                 st["deg_row"], start=True, stop=False,
                                 skip_group_check=True)

            # ---------------- messages (uniform full-width) ----------------
            for oi, off in enumerate(OFFSETS):
                for d in range(2):  # 0=fwd (src=t+off), 1=rev (src=t-off)
                    qo = off if d == 0 else -off
                    eo = 0 if d == 0 else -off
                    tz = wk.tile([128, 2048], bf16, name="tz", tag="tz")
                    nc.vector.tensor_add(
                        bass.AP(tensor=tz.tensor, offset=tz.offset,
                                ap=[[2048, 128], [512, 4], [1, 512]]),
                        bass.AP(tensor=P.tensor, offset=P.offset,
                                ap=[[2048, 128], [512, 4], [1, 512]]),
                        bass.AP(tensor=Q.tensor, offset=Q.offset + 8 + qo,
                                ap=[[2064, 128], [512, 4], [1, 512]]))
                    ew = e20[:, 8 + eo:8 + eo + 512]
                    for m in range(4):
                        blk = ((oi * 2 + d) * 4 + m) * 128
                        ze = pze.tile([128, 512], f32, name="ze", tag="ze")
                        nc.tensor.matmul(ze, CF[:, blk:blk + 128], ew,
                                         start=True, stop=False)
                        nc.tensor.matmul(ze, identb,
                                         tz[:, m * 512:(m + 1) * 512],
                                         start=False, stop=True)
                        tzg = wk.tile([128, 512], bf16, name="tzg",
                                      tag="tzg")
                        nc.scalar.activation(tzg, ze, AF.Gelu)
                        for fh in range(2):
                            last = (oi == 3 and d == 1 and m == 3
                                    and fh == 1)
                            nc.tensor.matmul(
                                agg[:, fh * 512:(fh + 1) * 512],
                                W2[:, m * 256 + fh * 128:
                                   m * 256 + (fh + 1) * 128],
                                tzg, start=False, stop=last,
                                skip_group_check=True)

            # ---------------- update MLP ----------------
            h2 = wk.tile([128, 1024], bf16, name="h2", tag="h2", bufs=1)
            nc.vector.tensor_add(h2, hfm, agg)
            hun = ln_fm(h2, lnuw, lnub, "u")
            gu = wk.tile([128, 4096], bf16, name="gu", tag="gu", bufs=1)
            for um in range(8):
                up = pmm.tile([128, 512], f32, name="mmps", tag="mmps")
                nc.tensor.matmul(up, U1a[:, um * 128:(um + 1) * 128],
                                 hun[:, 0:512], start=True, stop=False)
                nc.tensor.matmul(up, U1b[:, um * 128:(um + 1) * 128],
                                 hun[:, 512:1024], start=False, stop=True)
                nc.scalar.activation(gu[:, um * 512:(um + 1) * 512],
                                     up, AF.Gelu, bias=bu1c[:, um:um + 1])
            of = wk.tile([128, 1024], f32, name="of", tag="of", bufs=1)
            for fh in range(2):
                u2p = pmm.tile([128, 512], f32, name="mmps", tag="mmps")
                for k in range(8):
                    nc.tensor.matmul(
                        u2p,
                        U2[:, k * 256 + fh * 128:k * 256 + (fh + 1) * 128],
                        gu[:, k * 512:(k + 1) * 512],
                        start=(k == 0), stop=(k == 7))
                nc.vector.scalar_tensor_tensor(
                    out=of[:, fh * 512:(fh + 1) * 512], in0=u2p,
                    scalar=bu2c[:, fh:fh + 1],
                    in1=h2[:, fh * 512:(fh + 1) * 512],
                    op0=ALU.add, op1=ALU.add)

            # ------------- transpose out + mask + store (token-major) -------
            osb = ld.tile([128, 1024], f32, name="osb", tag="osb")
            for g in range(4):
                for fh in range(2):
                    tp = pze.tile([128, 128], f32, name="tpo", tag="ze")
                    nc.tensor.transpose(
                        tp,
                        of[:, fh * 512 + g * 128:fh * 512 + (g + 1) * 128],
                        ident)
                    nc.scalar.activation(
                        osb[:, g * 256 + fh * 128:g * 256 + (fh + 1) * 128],
                        tp, AF.Copy, bias=0.0, scale=vcolf[:, g:g + 1])
            ob = out_d[ci]
            nc.sync.dma_start(
                bass.AP(tensor=ob.tensor, offset=ob.offset,
                        ap=[[256, 128], [128 * 256, 4], [1, 256]]),
                osb.rearrange("p (i f) -> p i f", i=4))

        # 2-stage software pipeline: stage_a(ci+1) is emitted before
        # stage_b(ci) so its PE-light prep work fills the gaps of the
        # previous chunk's dense message/update phase.
        st = stage_a(0)
        states = {0: st}
        for ci in range(n_chunks):
            if ci + 1 < n_chunks:
                states[ci + 1] = stage_a(ci + 1)
            stage_b(ci, states.pop(ci))

    return nc


def _get_nc(n_chunks):
    if n_chunks not in _nc_cache:
        _nc_cache[n_chunks] = _build(n_chunks)
    return _nc_cache[n_chunks]


# ---------------------------------------------------------------------------
# Host-side constant prep
# ---------------------------------------------------------------------------
def _prep_consts(w1, b1, w2, b2, ln_n_w, ln_n_b, u1, bu1, u2, bu2,
                 ln_u_w, ln_u_b):
    f = np.float32
    w1 = np.asarray(w1, f)
    A = np.ascontiguousarray(
        np.stack([w1[0:128, :], w1[128:256, :]])).astype(BF)
    B = np.ascontiguousarray(
        np.stack([w1[256:384, :], w1[384:512, :]])).astype(BF)
    W2 = np.zeros((128, 1024), BF)
    for k in range(4):
        W2[:, k * 256:(k + 1) * 256] = np.asarray(
            w2[k * 128:(k + 1) * 128, :], f).astype(BF)
    U1 = np.ascontiguousarray(
        np.stack([np.asarray(u1, f)[0:128, :],
                  np.asarray(u1, f)[128:256, :]])).astype(BF)
    U2 = np.zeros((128, 2048), BF)
    for k in range(8):
        U2[:, k * 256:(k + 1) * 256] = np.asarray(
            u2[k * 128:(k + 1) * 128, :], f).astype(BF)
    Cu = w1[512:515, :]          # [3, 512]
    Cd = w1[515, :]              # [512]
    b1f = np.asarray(b1, f)      # [512]
    CF = np.zeros((128, 4096), f)
    for oi in range(4):
        for d in range(2):
            sgn = 1.0 if d == 0 else -1.0
            for m in range(4):
                blk = ((oi * 2 + d) * 4 + m) * 128
                CF[3 * oi:3 * oi + 3, blk:blk + 128] = \
                    sgn * Cu[:, m * 128:(m + 1) * 128]
                CF[32 + oi, blk:blk + 128] = Cd[m * 128:(m + 1) * 128]
                CF[64 + oi, blk:blk + 128] = BIG
                CF[96, blk:blk + 128] = b1f[m * 128:(m + 1) * 128]
    return dict(
        A=A, B=B, W2=W2, U1=U1, U2=U2, CF=CF.astype(BF),
        bu1c=np.ascontiguousarray(np.asarray(bu1, f).reshape(8, 128).T),
        bu2c=np.ascontiguousarray(np.asarray(bu2, f).reshape(2, 128).T),
        b2r=np.asarray(b2, f).reshape(1, 256).astype(BF),
        lnw=np.asarray(ln_n_w, f).reshape(1, 256).astype(BF),
        lnb=np.asarray(ln_n_b, f).reshape(1, 256).astype(BF),
        lnuw=np.asarray(ln_u_w, f).reshape(1, 256).astype(BF),
        lnub=np.asarray(ln_u_b, f).reshape(1, 256).astype(BF),
        onesH=np.full((128, 1), 1.0 / H, f).astype(BF),
        ones4=np.ones((4, 1), BF),
        seldsq=_seldsq(), selbc=_selbc(),
        onesr=np.ones((1, 512), BF),
        ident=np.eye(128, dtype=f),
        identb=np.eye(128, dtype=f).astype(BF),
        seed=np.full((128, 24), MAGIC, np.int32),
    )


def _seldsq():
    s = np.zeros((12, 4), np.float32)
    for o in range(4):
        s[3 * o:3 * o + 3, o] = 1.0
    return s.astype(BF)


def _selbc():
    s = np.zeros((4, 12), np.float32)
    for o in range(4):
        s[o, 3 * o:3 * o + 3] = 1.0
    return s.astype(BF)


def _run(h, xyz, valid, consts, n_chunks_per_core, core_ids, trace=False):
    from concourse.bass_utils import run_bass_kernel_spmd

    _install_patch()
    nc = _get_nc(n_chunks_per_core)
    ncore = len(core_ids)
    in_maps = []
    for i in range(ncore):
        s = slice(i * n_chunks_per_core, (i + 1) * n_chunks_per_core)
        im = dict(consts)
        im["h"] = np.ascontiguousarray(h[s])
        im["xyz"] = np.ascontiguousarray(xyz[s])
        im["valid"] = np.ascontiguousarray(valid[s])
        in_maps.append(im)
    res = run_bass_kernel_spmd(nc, in_maps, core_ids=core_ids, trace=trace)
    outs = [res.results[i]["out"] for i in range(ncore)]
    return np.concatenate(outs, axis=0), res


def kernel(h, xyz, valid, ln_n_w, ln_n_b, w1, b1, w2, b2,
           ln_u_w, ln_u_b, u1, bu1, u2, bu2):
    h = np.asarray(h, np.float32)
    xyz = np.asarray(xyz, np.float32)
    valid = np.asarray(valid).astype(np.uint8)
    consts = _prep_consts(
        np.asarray(w1), np.asarray(b1), np.asarray(w2), np.asarray(b2),
        np.asarray(ln_n_w), np.asarray(ln_n_b), np.asarray(u1),
        np.asarray(bu1), np.asarray(u2), np.asarray(bu2),
        np.asarray(ln_u_w), np.asarray(ln_u_b))
    out, _ = _run(h, xyz, valid, consts, CPC, list(range(N_CORES)))
    return out.astype(np.float32)


# revision 36
# speedup vs baseline: 1.8345x; 1.8345x over previous
"""LocalGraphMessageBlock TRN2 kernel (v2).

Math (per chunk of C=512 tokens, H=256 features, offsets 1,2,4,8):
  h_in = LN(h);  per offset o and direction, uniform full-width N=512:
    z = P[t] + Q[t+qo] + CF @ e20[t+eo]   (qo=+o fwd / -o rev, eo=0 / -o)
    CF carries: +-C_unit, C_dist, BIG*(mask-1) and b1 (via const ones row),
    so gelu(z)=0 exactly on invalid/out-of-range edges.
    agg += gelu(z) @ w2            (PSUM accumulation across all 8 dirs)
  agg += b2 (x) deg                (outer product, deg = #valid edges per dst)
  h2 = h + agg;  out = (h2 + MLP(LN(h2))) * valid

Implementation notes:
  - Feature-major on chip ([128 feat, 512 tok]); tokens enter/leave via
    transpose-DMA access patterns (512B descriptors), no PE transposes.
  - z assembled in PSUM: e-matmul (start) + identity-matmul of tz (stop),
    where tz = P + Q_window is ONE [128,4,512] 3D-AP DVE add per dir-off.
    Gelu reads PSUM directly on ACT ([128,1024] per half).
  - P = A^T h_in, Q = B^T h_in precomputed per chunk (A,B = halves of w1);
    b1 folded into CF's const-ones row (e20 row 96).
  - rsqrt via DVE bit-hack + 1 Newton step (ACT stays on the gelu table).
  - PSUM: agg 2 banks + ze 2x2 banks + mmps 1 + rows 1 = 8.

Data-parallel over the chunk dim N: 256 chunks / 8 cores = 32 chunks each,
same NEFF, per-core input slices.
"""
import json

import numpy as np
import ml_dtypes

BF = ml_dtypes.bfloat16

N_TOT, C, H = 256, 512, 256
OFFSETS = (1, 2, 4, 8)
N_CORES = 8
CPC = N_TOT // N_CORES  # chunks per core
BIG = 30000.0
EPS = 1e-5
MAGIC = 0x5F3759DF

# ---------------------------------------------------------------------------
# Walrus workaround: this container's walrus accepts at most ONE sync-wait
# command per instruction; Tile emits more. Split excess onto preceding
# NoOps on the same engine (engine queues are in-order, so this is
# equivalent gating).
# ---------------------------------------------------------------------------
_patched = False


def _split_sync_waits(bir_json: bytes, maxw: int = 1) -> bytes:
    m = json.loads(bir_json)
    cnt = 0
    changed = False
    for f in m.get("functions", []):
        for blk in f.get("blocks", []):
            newins = []
            for ins in blk.get("instructions", []):
                si = ins.get("sync_info")
                if si:
                    waits = si.get("on_wait") or []
                    if len(waits) > maxw:
                        changed = True
                        si["on_wait"] = waits[-maxw:]
                        extra = waits[:-maxw]
                        for i in range(0, len(extra), maxw):
                            cnt += 1
                            newins.append({
                                "debug": ins.get("debug", 0),
                                "engine": ins["engine"],
                                "ins": [], "outs": [],
                                "name": f"{ins['name']}-ws{cnt}",
                                "opcode": "NoOp",
                                "sync_info": {"on_update": [],
                                              "on_wait": extra[i:i + maxw]},
                            })
                newins.append(ins)
            blk["instructions"] = newins
    return json.dumps(m).encode() if changed else bir_json


def _install_patch():
    global _patched
    if _patched:
        return
    import concourse.bass_utils as bu
    import concourse.bass2jax as b2j

    orig = bu.compile_bir_kernel

    def patched(bir_json, tmpdir, neff_name="file.neff"):
        return orig(_split_sync_waits(bir_json), tmpdir, neff_name)

    bu.compile_bir_kernel = patched
    b2j.compile_bir_kernel = patched
    _patched = True


# ---------------------------------------------------------------------------
# Bass kernel builder
# ---------------------------------------------------------------------------
_nc_cache = {}


def _build(n_chunks):
    import concourse.bass as bass
    import concourse.tile as tile
    from concourse import mybir

    f32 = mybir.dt.float32
    bf16 = mybir.dt.bfloat16
    f8 = mybir.dt.float8e4
    i32 = mybir.dt.int32
    u8 = mybir.dt.uint8
    AF = mybir.ActivationFunctionType
    ALU = mybir.AluOpType
    DR = mybir.MatmulPerfMode.DoubleRow

    nc = bass.Bass("TRN2")

    # ---- dram I/O ----
    h_d = nc.dram_tensor("h", [n_chunks, C, H], f32, kind="ExternalInput")
    xyz_d = nc.dram_tensor("xyz", [n_chunks, C, 3], f32, kind="ExternalInput")
    val_d = nc.dram_tensor("valid", [n_chunks, C], u8, kind="ExternalInput")
    out_d = nc.dram_tensor("out", [n_chunks, C, H], f32, kind="ExternalOutput")

    def din(name, shape, dt=None):
        return nc.dram_tensor(name, shape, dt or bf16, kind="ExternalInput")

    A_d = din("A", [2, 128, 512])       # w1[:256] k-halves  (lhsT blocks)
    B_d = din("B", [2, 128, 512])       # w1[256:512]
    W2_d = din("W2", [128, 1024])       # w2 k-blocks: [:, k*256:(k+1)*256]
    # fp8 DoubleRow layouts: U1 [:, um*256 + j*128 + c]  (j = hun half)
    # U2 [:, (q*2+fh)*256 + j*128 + o]  (q = k-pair)
    U1_d = din("U1", [128, 2048], f8)
    U2_d = din("U2", [128, 2048], f8)
    CF_d = din("CF", [128, 4096])       # e-matmul lhsT blocks (oi,dir,m)
    bu1c_d = din("bu1c", [128, 8], f32)
    bu2c_d = din("bu2c", [128, 2], f32)
    b2r_d = din("b2r", [1, 256])
    lnw_d = din("lnw", [1, 256])
    lnb_d = din("lnb", [1, 256])
    lnuw_d = din("lnuw", [1, 256])
    lnub_d = din("lnub", [1, 256])
    onesH_d = din("onesH", [128, 1])    # 1/H
    ones4_d = din("ones4", [4, 1])
    seldsq_d = din("seldsq", [12, 4])
    selbc_d = din("selbc", [4, 12])
    onesr_d = din("onesr", [1, 512])
    ident_d = din("ident", [128, 128], f32)
    identb_d = din("identb", [128, 128])
    seed_d = din("seed", [128, 24], i32)

    from contextlib import ExitStack
    with tile.TileContext(nc) as tc, ExitStack() as ctx:
        cp = ctx.enter_context(tc.tile_pool(name="consts", bufs=1))
        ld = ctx.enter_context(tc.tile_pool(name="loads", bufs=2))
        wk = ctx.enter_context(tc.tile_pool(name="work", bufs=2))
        sm = ctx.enter_context(tc.tile_pool(name="small", bufs=2))
        pagg = ctx.enter_context(tc.tile_pool(name="pagg", bufs=1,
                                              space="PSUM"))
        pze = ctx.enter_context(tc.tile_pool(name="pze", bufs=2,
                                             space="PSUM"))
        pmm = ctx.enter_context(tc.tile_pool(name="pmm", bufs=2, space="PSUM"))
        prw = ctx.enter_context(tc.tile_pool(name="prw", bufs=2, space="PSUM"))

        # ---- load constants ----
        def cload(dram, shape, dt=None, name=None):
            t = cp.tile(shape, dt or bf16, name=name, tag=name)
            nc.sync.dma_start(t, dram[tuple(slice(None) for _ in shape)])
            return t

        A0 = cp.tile([128, 512], bf16); nc.sync.dma_start(A0, A_d[0])
        A1 = cp.tile([128, 512], bf16); nc.sync.dma_start(A1, A_d[1])
        B0 = cp.tile([128, 512], bf16); nc.sync.dma_start(B0, B_d[0])
        B1 = cp.tile([128, 512], bf16); nc.sync.dma_start(B1, B_d[1])
        W2 = cload(W2_d, [128, 1024], name="W2")
        U1 = cload(U1_d, [128, 2048], f8, name="U1")
        U2 = cload(U2_d, [128, 2048], f8, name="U2")
        CF = cload(CF_d, [128, 4096], name="CF")
        bu1c = cload(bu1c_d, [128, 8], f32, name="bu1c")
        bu2c = cload(bu2c_d, [128, 2], f32, name="bu2c")
        b2r = cload(b2r_d, [1, 256], name="b2r")
        lnw = cload(lnw_d, [1, 256], name="lnw")
        lnb = cload(lnb_d, [1, 256], name="lnb")
        lnuw = cload(lnuw_d, [1, 256], name="lnuw")
        lnub = cload(lnub_d, [1, 256], name="lnub")
        onesH = cload(onesH_d, [128, 1], name="onesH")
        ones4 = cload(ones4_d, [4, 1], name="ones4")
        seldsq = cload(seldsq_d, [12, 4], name="seldsq")
        selbc = cload(selbc_d, [4, 12], name="selbc")
        onesr = cload(onesr_d, [1, 512], name="onesr")
        ident = cload(ident_d, [128, 128], f32, name="ident")
        identb = cload(identb_d, [128, 128], name="identb")
        seed = cload(seed_d, [128, 24], i32, name="seed")

        # e20 rows: unit 0:12 (3/oi), dist 32:36, mask 64:68, ones 96.
        # Two buffers (chunk parity). Init all to -1 (pad cols' mask=-1 ->
        # z-=BIG -> gelu=0; dead rows hit zero CF rows), then ones row.
        e20s = []
        for pbuf in range(2):
            e = cp.tile([128, 520], bf16, name=f"e20_{pbuf}", tag=f"e20_{pbuf}")
            nc.vector.tensor_scalar(out=e, in0=CF[:, 0:520],
                                    scalar1=0.0, scalar2=-1.0,
                                    op0=ALU.mult, op1=ALU.add)
            nc.vector.memset(e[96:97], 1.0)
            e20s.append(e)
        Qs = []
        for pbuf in range(2):
            q = cp.tile([128, 2064], bf16, name=f"Qt_{pbuf}", tag=f"Qt_{pbuf}")
            nc.vector.memset(q[:, 0:8], 0.0)
            nc.vector.memset(q[:, 2056:2064], 0.0)
            Qs.append(q)

        def rsqrt_rows(rows, r, tag, want_f32=False):
            """rows: [r, 512] f32 sbuf (positive) -> [r, 512] bf16 1/sqrt
            (optionally also f32) via bit-hack + 1 Newton step."""
            w = 4 * r
            rt = prw.tile([128, w], f32, name="rt", tag="rows")
            for g in range(4):
                nc.tensor.transpose(rt[:, g * r:(g + 1) * r],
                                    rows[:, g * 128:(g + 1) * 128],
                                    ident[0:r, 0:r])
            x = sm.tile([128, w], f32, name=f"nrx{tag}", tag=f"nrx{tag}")
            nc.vector.tensor_copy(x, rt)
            yi = sm.tile([128, w], i32, name=f"nry{tag}", tag=f"nry{tag}")
            nc.vector.tensor_scalar(out=yi, in0=x.bitcast(i32), scalar1=1,
                                    scalar2=None, op0=ALU.logical_shift_right)
            nc.vector.tensor_sub(yi, seed[:, 0:w], yi)
            y0 = yi.bitcast(f32)
            y = sm.tile([128, w], f32, name=f"nryy{tag}", tag=f"nryy{tag}")
            t = sm.tile([128, w], f32, name=f"nrt{tag}", tag=f"nrt{tag}")
            nc.vector.tensor_mul(t, y0, y0)
            nc.vector.tensor_mul(t, t, x)
            nc.vector.tensor_scalar(out=t, in0=t, scalar1=-0.5,
                                    scalar2=1.5, op0=ALU.mult, op1=ALU.add)
            nc.vector.tensor_mul(y, y0, t)
            rp = prw.tile([r, 512], f32, name="rp", tag="rows")
            for g in range(4):
                nc.tensor.transpose(rp[:, g * 128:(g + 1) * 128],
                                    y[:, g * r:(g + 1) * r], ident)
            outb = sm.tile([r, 512], bf16, name=f"nro{tag}", tag=f"nro{tag}")
            nc.vector.tensor_copy(outb, rp)
            if not want_f32:
                return outb, None
            outf = sm.tile([r, 512], f32, name=f"nrof{tag}", tag=f"nrof{tag}")
            nc.vector.tensor_copy(outf, rp)
            return outb, outf

        def ln_fm(hfm, wrow, brow, tag, out_dt=None):
            """Feature-major layernorm of hfm [128,1024] bf16 -> [128,1024]
            bf16 (fh blocks of 512 tokens side by side)."""
            mu_ps = prw.tile([1, 512], f32, name="mu_ps", tag="rows")
            m2_ps = prw.tile([1, 512], f32, name="m2_ps", tag="rows")
            x2 = wk.tile([128, 1024], bf16, name=f"x2{tag}", tag="x2", bufs=1)
            nc.gpsimd.tensor_mul(x2, hfm, hfm)
            for fh in range(2):
                nc.tensor.matmul(mu_ps, onesH, hfm[:, fh * 512:
                                                   (fh + 1) * 512],
                                 start=(fh == 0), stop=(fh == 1))
                nc.tensor.matmul(m2_ps, onesH, x2[:, fh * 512:
                                                  (fh + 1) * 512],
                                 start=(fh == 0), stop=(fh == 1))
            mu_row = sm.tile([1, 512], f32, name=f"mur{tag}", tag=f"mur{tag}")
            nc.scalar.copy(mu_row, mu_ps)
            mumu = sm.tile([1, 512], f32, name=f"mumu{tag}", tag=f"mumu{tag}")
            nc.scalar.activation(mumu, mu_ps, AF.Square)
            vare = sm.tile([1, 512], f32, name=f"var{tag}", tag=f"var{tag}")
            nc.vector.scalar_tensor_tensor(out=vare, in0=m2_ps,
                                           scalar=EPS, in1=mumu, op0=ALU.add,
                                           op1=ALU.subtract)
            rstd, _ = rsqrt_rows(vare, 1, tag)
            sh_row = sm.tile([1, 512], bf16, name=f"shr{tag}", tag=f"shr{tag}")
            nc.vector.scalar_tensor_tensor(out=sh_row, in0=mu_row, scalar=-1.0,
                                           in1=rstd, op0=ALU.mult,
                                           op1=ALU.mult)
            o = wk.tile([128, 1024], out_dt or bf16, name=f"ln{tag}",
                        tag=f"ln{tag}", bufs=2)
            for fh in range(2):
                arep = prw.tile([128, 512], f32, name="arep", tag="rows")
                nc.tensor.matmul(arep, wrow[:, fh * 128:(fh + 1) * 128], rstd,
                                 start=True, stop=True)
                brep = prw.tile([128, 512], f32, name="brep", tag="rows")
                nc.tensor.matmul(brep, wrow[:, fh * 128:(fh + 1) * 128],
                                 sh_row, start=True, stop=False)
                nc.tensor.matmul(brep, brow[:, fh * 128:(fh + 1) * 128],
                                 onesr, start=False, stop=True)
                sl = o[:, fh * 512:(fh + 1) * 512]
                nc.vector.tensor_mul(sl, hfm[:, fh * 512:(fh + 1) * 512],
                                     arep)
                nc.vector.tensor_add(sl, sl, brep)
            return o

        def stage_a(ci):
            """Loads, in-transposes, LN1, edge features, P/Q. PE-light —
            scheduled to overlap the previous chunk's stage_b."""
            e20 = e20s[ci % 2]
            Q = Qs[ci % 2]
            # ---------------- loads (token-major h) ----------------
            ht = ld.tile([128, 1024], f32, name="ht", tag="ht")
            hb = h_d[ci]
            nc.sync.dma_start(
                ht.rearrange("p (i f) -> p i f", i=4),
                bass.AP(tensor=hb.tensor, offset=hb.offset,
                        ap=[[256, 128], [128 * 256, 4], [1, 256]]))
            xyzp = sm.tile([3, 520], f32, name="xyzp", tag="xyzp")
            nc.vector.memset(xyzp, 0.0)
            nc.sync.dma_start(xyzp[:, 0:512],
                              xyz_d[ci].rearrange("t k -> k t"))
            vbase = val_d[ci]
            vr_u8 = sm.tile([4, 512], u8, name="vru", tag="vru")
            nc.sync.dma_start(
                vr_u8, bass.AP(tensor=vbase.tensor, offset=vbase.offset,
                               ap=[[0, 4], [1, 512]]))
            vrf = sm.tile([4, 512], f32, name="vrf", tag="vrf")
            nc.vector.tensor_copy(vrf, vr_u8)
            vrs_u8 = sm.tile([4, 512], u8, name="vrsu", tag="vrsu")
            nc.vector.memset(vrs_u8, 0)
            for oi, off in enumerate(OFFSETS):
                nc.sync.dma_start(vrs_u8[oi:oi + 1, 0:C - off],
                                  val_d[ci, off:C][None, :])
            vrsf = sm.tile([4, 512], f32, name="vrsf", tag="vrsf")
            nc.vector.tensor_copy(vrsf, vrs_u8)
            vrp_u8 = sm.tile([4, 512], u8, name="vrpu", tag="vrpu")
            nc.vector.memset(vrp_u8, 0)
            for oi, off in enumerate(OFFSETS):
                nc.sync.dma_start(vrp_u8[oi:oi + 1, off:C],
                                  val_d[ci, 0:C - off][None, :])
            vrpf = sm.tile([4, 512], f32, name="vrpf", tag="vrpf")
            nc.vector.tensor_copy(vrpf, vrp_u8)
            vcol_u8 = sm.tile([128, 4], u8, name="vcu", tag="vcu")
            nc.sync.dma_start(
                vcol_u8, bass.AP(tensor=vbase.tensor, offset=vbase.offset,
                                 ap=[[1, 128], [128, 4]]))
            vcolf = sm.tile([128, 4], f32, name="vcf", tag="vcf")
            nc.vector.tensor_copy(vcolf, vcol_u8)

            # ---------------- h -> feature-major (bf16) ----------------
            hfm = wk.tile([128, 1024], bf16, name="hfm", tag="hfm")
            for g in range(4):
                for fh in range(2):
                    tp = prw.tile([128, 128], f32, name="tp", tag="rows")
                    nc.tensor.transpose(
                        tp,
                        ht[:, g * 256 + fh * 128:g * 256 + (fh + 1) * 128],
                        ident)
                    nc.scalar.copy(
                        hfm[:, fh * 512 + g * 128:fh * 512 + (g + 1) * 128],
                        tp)

            # ---------------- LN1 ----------------
            hin = ln_fm(hfm, lnw, lnb, "a")

            # ---------------- edge features ----------------
            delta = sm.tile([12, 512], f32, name="delta", tag="delta")
            for oi, off in enumerate(OFFSETS):
                dlo = sm.tile([3, 512], f32, name=f"dlo{oi}", tag=f"dlo{oi}")
                nc.vector.tensor_sub(dlo, xyzp[:, off:off + 512],
                                     xyzp[:, 0:512])
                nc.sync.dma_start(delta[3 * oi:3 * oi + 3], dlo)
            dsq = sm.tile([12, 512], bf16, name="dsq", tag="dsq")
            nc.gpsimd.tensor_mul(dsq, delta, delta)
            d2_ps = prw.tile([4, 512], f32, name="d2_ps", tag="rows")
            nc.tensor.matmul(d2_ps, seldsq, dsq, start=True, stop=True)
            R = sm.tile([4, 512], f32, name="Rrows", tag="Rrows")
            nc.vector.tensor_scalar(out=R[0:4], in0=d2_ps,
                                    scalar1=1e-12, scalar2=None, op0=ALU.max)

            m_all = sm.tile([4, 512], bf16, name="mall", tag="mall")
            nc.gpsimd.tensor_mul(m_all, vrf, vrsf)
            nc.vector.tensor_scalar(out=e20[64:68, 8:520], in0=m_all,
                                    scalar1=1.0, scalar2=None,
                                    op0=ALU.subtract)

            rsq, rsqf = rsqrt_rows(R, 4, "e", want_f32=True)

            invrep = prw.tile([12, 512], f32, name="invrep", tag="rows")
            nc.tensor.matmul(invrep, selbc, rsq[0:4], start=True, stop=True)
            nc.vector.tensor_mul(e20[0:12, 8:520], delta, invrep)
            nc.vector.tensor_mul(e20[32:36, 8:520], R[0:4], rsqf[0:4])

            # ---------------- degree / agg init ----------------
            mrev = sm.tile([4, 512], bf16, name="mrev", tag="mrev")
            nc.gpsimd.tensor_mul(mrev, vrf, vrpf)
            deg_ps = prw.tile([1, 512], f32, name="deg_ps", tag="rows")
            nc.tensor.matmul(deg_ps, ones4, m_all, start=True, stop=False)
            nc.tensor.matmul(deg_ps, ones4, mrev, start=False, stop=True)
            deg_row = sm.tile([1, 512], bf16, name="degr", tag="degr")
            nc.scalar.copy(deg_row, deg_ps)

            # ---------------- P, Q ----------------
            P = wk.tile([128, 2048], bf16, name="P", tag="P")
            for m in range(4):
                pq = pmm.tile([128, 512], f32, name="mmps", tag="mmps")
                nc.tensor.matmul(pq, A0[:, m * 128:(m + 1) * 128],
                                 hin[:, 0:512], start=True, stop=False)
                nc.tensor.matmul(pq, A1[:, m * 128:(m + 1) * 128],
                                 hin[:, 512:1024], start=False, stop=True)
                nc.vector.tensor_copy(P[:, m * 512:(m + 1) * 512], pq)
                pq2 = pmm.tile([128, 512], f32, name="mmps", tag="mmps")
                nc.tensor.matmul(pq2, B0[:, m * 128:(m + 1) * 128],
                                 hin[:, 0:512], start=True, stop=False)
                nc.tensor.matmul(pq2, B1[:, m * 128:(m + 1) * 128],
                                 hin[:, 512:1024], start=False, stop=True)
                nc.vector.tensor_copy(Q[:, 8 + m * 512:8 + (m + 1) * 512],
                                      pq2)
            return dict(hfm=hfm, P=P, deg_row=deg_row, vcolf=vcolf)

        def stage_b(ci, st):
            """agg init, message loop, update MLP, masked store."""
            e20 = e20s[ci % 2]
            Q = Qs[ci % 2]
            hfm = st["hfm"]
            P = st["P"]
            vcolf = st["vcolf"]

            agg = pagg.tile([128, 1024], f32, name="agg", tag="agg")
            for fh in range(2):
                nc.tensor.matmul(agg[:, fh * 512:(fh + 1) * 512],
                                 b2r[:, fh * 128:(fh + 1) * 128],
                                 st["deg_row"], start=True, stop=False,
                                 skip_group_check=True)

            # ---------------- messages (uniform full-width) ----------------
            for oi, off in enumerate(OFFSETS):
                for d in range(2):  # 0=fwd (src=t+off), 1=rev (src=t-off)
                    qo = off if d == 0 else -off
                    eo = 0 if d == 0 else -off
                    tz = wk.tile([128, 2048], bf16, name="tz", tag="tz")
                    nc.vector.tensor_add(
                        bass.AP(tensor=tz.tensor, offset=tz.offset,
                                ap=[[2048, 128], [512, 4], [1, 512]]),
                        bass.AP(tensor=P.tensor, offset=P.offset,
                                ap=[[2048, 128], [512, 4], [1, 512]]),
                        bass.AP(tensor=Q.tensor, offset=Q.offset + 8 + qo,
                                ap=[[2064, 128], [512, 4], [1, 512]]))
                    ew = e20[:, 8 + eo:8 + eo + 512]
                    for m in range(4):
                        blk = ((oi * 2 + d) * 4 + m) * 128
                        ze = pze.tile([128, 512], f32, name="ze", tag="ze")
                        nc.tensor.matmul(ze, CF[:, blk:blk + 128], ew,
                                         start=True, stop=False)
                        nc.tensor.matmul(ze, identb,
                                         tz[:, m * 512:(m + 1) * 512],
                                         start=False, stop=True)
                        tzg = wk.tile([128, 512], bf16, name="tzg",
                                      tag="tzg")
                        nc.scalar.activation(tzg, ze, AF.Gelu)
                        for fh in range(2):
                            last = (oi == 3 and d == 1 and m == 3
                                    and fh == 1)
                            nc.tensor.matmul(
                                agg[:, fh * 512:(fh + 1) * 512],
                                W2[:, m * 256 + fh * 128:
                                   m * 256 + (fh + 1) * 128],
                                tzg, start=False, stop=last,
                                skip_group_check=True)

            # ---------------- update MLP ----------------
            h2 = wk.tile([128, 1024], bf16, name="h2", tag="h2", bufs=1)
            nc.vector.tensor_add(h2, hfm, agg)
            hun = ln_fm(h2, lnuw, lnub, "u", out_dt=f8)
            hun_dr = bass.AP(tensor=hun.tensor, offset=hun.offset,
                             ap=[[1024, 128], [512, 2], [1, 512]])
            gu = wk.tile([128, 4096], f8, name="gu", tag="gu", bufs=1)
            for um in range(8):
                up = pmm.tile([128, 512], f32, name="mmps", tag="mmps")
                nc.tensor.matmul(
                    up,
                    bass.AP(tensor=U1.tensor, offset=U1.offset + um * 256,
                            ap=[[2048, 128], [128, 2], [1, 128]]),
                    hun_dr, perf_mode=DR, start=True, stop=True)
                nc.scalar.activation(gu[:, um * 512:(um + 1) * 512],
                                     up, AF.Gelu, bias=bu1c[:, um:um + 1])
            of = wk.tile([128, 1024], f32, name="of", tag="of", bufs=1)
            for fh in range(2):
                u2p = pmm.tile([128, 512], f32, name="mmps", tag="mmps")
                for q in range(4):
                    nc.tensor.matmul(
                        u2p,
                        bass.AP(tensor=U2.tensor,
                                offset=U2.offset + (q * 2 + fh) * 256,
                                ap=[[2048, 128], [128, 2], [1, 128]]),
                        bass.AP(tensor=gu.tensor, offset=gu.offset + q * 1024,
                                ap=[[4096, 128], [512, 2], [1, 512]]),
                        perf_mode=DR, start=(q == 0), stop=(q == 3))
                nc.vector.scalar_tensor_tensor(
                    out=of[:, fh * 512:(fh + 1) * 512], in0=u2p,
                    scalar=bu2c[:, fh:fh + 1],
                    in1=h2[:, fh * 512:(fh + 1) * 512],
                    op0=ALU.add, op1=ALU.add)

            # ------------- transpose out + mask + store (token-major) -------
            osb = ld.tile([128, 1024], f32, name="osb", tag="osb")
            for g in range(4):
                for fh in range(2):
                    tp = pze.tile([128, 128], f32, name="tpo", tag="ze")
                    nc.tensor.transpose(
                        tp,
                        of[:, fh * 512 + g * 128:fh * 512 + (g + 1) * 128],
                        ident)
                    nc.scalar.activation(
                        osb[:, g * 256 + fh * 128:g * 256 + (fh + 1) * 128],
                        tp, AF.Copy, bias=0.0, scale=vcolf[:, g:g + 1])
            ob = out_d[ci]
            nc.sync.dma_start(
                bass.AP(tensor=ob.tensor, offset=ob.offset,
                        ap=[[256, 128], [128 * 256, 4], [1, 256]]),
                osb.rearrange("p (i f) -> p i f", i=4))

        # 2-stage software pipeline: stage_a(ci+1) is emitted before
        # stage_b(ci) so its PE-light prep work fills the gaps of the
        # previous chunk's dense message/update phase.
        st = stage_a(0)
        states = {0: st}
        for ci in range(n_chunks):
            if ci + 1 < n_chunks:
                states[ci + 1] = stage_a(ci + 1)
            stage_b(ci, states.pop(ci))

    return nc


def _get_nc(n_chunks):
    if n_chunks not in _nc_cache:
        _nc_cache[n_chunks] = _build(n_chunks)
    return _nc_cache[n_chunks]


# ---------------------------------------------------------------------------
# Host-side constant prep
# ---------------------------------------------------------------------------
def _prep_consts(w1, b1, w2, b2, ln_n_w, ln_n_b, u1, bu1, u2, bu2,
                 ln_u_w, ln_u_b):
    f = np.float32
    w1 = np.asarray(w1, f)
    A = np.ascontiguousarray(
        np.stack([w1[0:128, :], w1[128:256, :]])).astype(BF)
    B = np.ascontiguousarray(
        np.stack([w1[256:384, :], w1[384:512, :]])).astype(BF)
    W2 = np.zeros((128, 1024), BF)
    for k in range(4):
        W2[:, k * 256:(k + 1) * 256] = np.asarray(
            w2[k * 128:(k + 1) * 128, :], f).astype(BF)
    F8 = ml_dtypes.float8_e4m3
    u1f = np.asarray(u1, f)
    U1 = np.zeros((128, 2048), F8)
    for um in range(8):
        for j in range(2):
            U1[:, um * 256 + j * 128:um * 256 + (j + 1) * 128] = \
                u1f[j * 128:(j + 1) * 128,
                    um * 128:(um + 1) * 128].astype(F8)
    u2f = np.asarray(u2, f)
    U2 = np.zeros((128, 2048), F8)
    for q in range(4):
        for fh in range(2):
            for j in range(2):
                U2[:, (q * 2 + fh) * 256 + j * 128:
                   (q * 2 + fh) * 256 + (j + 1) * 128] = \
                    u2f[(2 * q + j) * 128:(2 * q + j + 1) * 128,
                        fh * 128:(fh + 1) * 128].astype(F8)
    Cu = w1[512:515, :]          # [3, 512]
    Cd = w1[515, :]              # [512]
    b1f = np.asarray(b1, f)      # [512]
    CF = np.zeros((128, 4096), f)
    for oi in range(4):
        for d in range(2):
            sgn = 1.0 if d == 0 else -1.0
            for m in range(4):
                blk = ((oi * 2 + d) * 4 + m) * 128
                CF[3 * oi:3 * oi + 3, blk:blk + 128] = \
                    sgn * Cu[:, m * 128:(m + 1) * 128]
                CF[32 + oi, blk:blk + 128] = Cd[m * 128:(m + 1) * 128]
                CF[64 + oi, blk:blk + 128] = BIG
                CF[96, blk:blk + 128] = b1f[m * 128:(m + 1) * 128]
    return dict(
        A=A, B=B, W2=W2, U1=U1, U2=U2, CF=CF.astype(BF),
        bu1c=np.ascontiguousarray(np.asarray(bu1, f).reshape(8, 128).T),
        bu2c=np.ascontiguousarray(np.asarray(bu2, f).reshape(2, 128).T),
        b2r=np.asarray(b2, f).reshape(1, 256).astype(BF),
        lnw=np.asarray(ln_n_w, f).reshape(1, 256).astype(BF),
        lnb=np.asarray(ln_n_b, f).reshape(1, 256).astype(BF),
        lnuw=np.asarray(ln_u_w, f).reshape(1, 256).astype(BF),
        lnub=np.asarray(ln_u_b, f).reshape(1, 256).astype(BF),
        onesH=np.full((128, 1), 1.0 / H, f).astype(BF),
        ones4=np.ones((4, 1), BF),
        seldsq=_seldsq(), selbc=_selbc(),
        onesr=np.ones((1, 512), BF),
        ident=np.eye(128, dtype=f),
        identb=np.eye(128, dtype=f).astype(BF),
        seed=np.full((128, 24), MAGIC, np.int32),
    )


def _seldsq():
    s = np.zeros((12, 4), np.float32)
    for o in range(4):
        s[3 * o:3 * o + 3, o] = 1.0
    return s.astype(BF)


def _selbc():
    s = np.zeros((4, 12), np.float32)
    for o in range(4):
        s[o, 3 * o:3 * o + 3] = 1.0
    return s.astype(BF)


def _run(h, xyz, valid, consts, n_chunks_per_core, core_ids, trace=False):
    from concourse.bass_utils import run_bass_kernel_spmd

    _install_patch()
    nc = _get_nc(n_chunks_per_core)
    ncore = len(core_ids)
    in_maps = []
    for i in range(ncore):
        s = slice(i * n_chunks_per_core, (i + 1) * n_chunks_per_core)
        im = dict(consts)
        im["h"] = np.ascontiguousarray(h[s])
        im["xyz"] = np.ascontiguousarray(xyz[s])
        im["valid"] = np.ascontiguousarray(valid[s])
        in_maps.append(im)
    res = run_bass_kernel_spmd(nc, in_maps, core_ids=core_ids, trace=trace)
    outs = [res.results[i]["out"] for i in range(ncore)]
    return np.concatenate(outs, axis=0), res


def kernel(h, xyz, valid, ln_n_w, ln_n_b, w1, b1, w2, b2,
           ln_u_w, ln_u_b, u1, bu1, u2, bu2):
    h = np.asarray(h, np.float32)
    xyz = np.asarray(xyz, np.float32)
    valid = np.asarray(valid).astype(np.uint8)
    consts = _prep_consts(
        np.asarray(w1), np.asarray(b1), np.asarray(w2), np.asarray(b2),
        np.asarray(ln_n_w), np.asarray(ln_n_b), np.asarray(u1),
        np.asarray(bu1), np.asarray(u2), np.asarray(bu2),
        np.asarray(ln_u_w), np.asarray(ln_u_b))
    out, _ = _run(h, xyz, valid, consts, CPC, list(range(N_CORES)))
    return out.astype(np.float32)
